# revision 9
# baseline (speedup 1.0000x reference)
"""Trainium2 Bass kernel for CustomMultiHeadAttention (B=4, S=1024, D=1024, H=16, Dh=64).

Sharding: 8 cores = (batch b in 0..3) x (head-group g in 0..1).
Core (b, g) computes heads 8g..8g+7 of batch b over the FULL sequence:
Q/K/V projections use only the group's 512 columns of Wq/Wk/Wv, the
output projection contracts the group's 512 rows of Wo, producing a
partial [S, D] output; the host sums the two partials per batch (+bo).
Nothing is computed twice across cores, and per-core input DMA drops
to ~6.7 MB.

Pipeline notes:
 - QT/KT rope via permutation-matmul + DVE; psum evac with fused bias
   on ScalarE (idle during the projection phase).
 - The causal mask is an additive PE matmul: identity^T @ (-240
   triangle) accumulated into the diagonal 128-col block of each score
   chunk before the exp, so no vector/gpsimd engine ever touches the
   mask (engine-queue head-of-line blocking killed a previous variant).
 - Attention runs as ONE flat (head, kv-block) software pipeline with
   ctx lagging scores by 4 steps, so the next head's score matmuls fill
   the PE queue while ctx waits on ScalarE exps; filler matmuls keep
   the HAM clock gate at K=8 through the ScalarE-bound stretch.
 - normalize: cx psum is copied to SBUF immediately (frees the psum
   bank for the next head), reciprocals on DVE, partition-broadcasts
   and the cn multiplies on GpSimd.
"""

import threading

import numpy as np

B, S, D, H, Dh = 4, 1024, 1024, 16, 64
P = 128
N_CORES = 8
NT = D // P        # 8 tiles along the model dim
HG = 8             # heads per core
QT_T = 4           # qt/kt dout tiles per core (2 heads each)
VS = 65            # V slot width: [V(64) | ones(1)] per head

_cache = {}
_lock = threading.Lock()


def _build_program(taps=False):
    import concourse.bass as bass  # noqa: F401
    import concourse.mybir as mybir
    import concourse.tile as tile
    from concourse import bacc

    dt = mybir.dt
    f16, f32 = dt.float16, dt.float32
    AF = mybir.ActivationFunctionType

    nc = bacc.Bacc("TRN2", target_bir_lowering=False, debug=False,
                   num_devices=N_CORES)

    def ein(name, shape):
        return nc.dram_tensor(name, shape, f16, kind="ExternalInput").ap()

    xt_e = ein("xt", [P, NT, S])          # x[b]^T, host-transposed
    wq_e = ein("wq", [P, NT, 512])        # Wq[:, half], host-tiled
    wk_e = ein("wk", [P, NT, 512])
    wv_e = ein("wv", [P, NT, 512])
    wo_e = ein("wo", [P, QT_T, D])        # Wo[half, :], host-tiled
    bqt_e = nc.dram_tensor("bqt", [P, QT_T], f32, kind="ExternalInput").ap()
    bkt_e = nc.dram_tensor("bkt", [P, QT_T], f32, kind="ExternalInput").ap()
    bvb_e = ein("bvb", [P, 512])          # bv[half] broadcast across parts
    cos_e = ein("cosk", [P, S])
    sin_e = ein("sink", [P, S])
    id_e = ein("id128", [P, P])           # identity (mask-add stationary)
    ma_e = ein("madd", [P, P])            # -240 strict-lower triangle
    p128_e = ein("p128", [P, P])          # rope xor-32 permutation
    y_e = nc.dram_tensor("y_sh", [S, D], f16, kind="ExternalOutput").ap()
    tap_ext = {}
    if taps:
        for tn, shape in (("qt", [P, QT_T, S]), ("kt", [P, QT_T, S]),
                          ("v1", [P, NT, HG * VS]), ("cn", [P, QT_T, S])):
            tap_ext[tn] = nc.dram_tensor("dbg_" + tn, shape, f16,
                                         kind="ExternalOutput").ap()

    with tile.TileContext(nc) as tc:
        from contextlib import ExitStack
        with ExitStack() as ctx:
            big = ctx.enter_context(tc.tile_pool(name="big", bufs=1))

            xT = big.tile([P, NT, S], f16, tag="xT")
            wq = big.tile([P, NT, 512], f16, tag="wq")
            wk = big.tile([P, NT, 512], f16, tag="wk")
            wv = big.tile([P, NT, 512], f16, tag="wv")
            wo = big.tile([P, QT_T, D], f16, tag="wo")
            bqt = big.tile([P, QT_T], f32, tag="bqt")
            bkt = big.tile([P, QT_T], f32, tag="bkt")
            bvb = big.tile([P, 512], f16, tag="bvb")
            qt = big.tile([P, QT_T, S], f16, tag="qt")    # rope'd Q^T
            kt = big.tile([P, QT_T, S], f16, tag="kt")    # rope'd K^T
            v1 = big.tile([P, NT, HG * VS], f16, tag="v1")
            cn = big.tile([P, QT_T, S], f16, tag="cn")    # normalized ctx^T
            cos = big.tile([P, S], f16, tag="cos")
            sin = big.tile([P, S], f16, tag="sin")
            id128 = big.tile([P, P], f16, tag="id128")
            madd = big.tile([P, P], f16, tag="madd")
            p128 = big.tile([P, P], f16, tag="p128")

            # ---- input DMAs ----
            # Three queues pull in parallel; per-queue order matches first
            # use. Every tensor is host-packed to >=2KB contiguous lines.
            for t, e in ((cos, cos_e), (sin, sin_e), (bqt, bqt_e),
                         (bkt, bkt_e), (p128, p128_e)):
                nc.scalar.dma_start(t[:], e[:])
            nc.gpsimd.dma_start(wq[:, 0:4, :], wq_e[:, 0:4, :])
            nc.sync.dma_start(xT[:, 0:4, :], xt_e[:, 0:4, :])
            nc.gpsimd.dma_start(wq[:, 4:8, :], wq_e[:, 4:8, :])
            nc.sync.dma_start(xT[:, 4:8, :], xt_e[:, 4:8, :])
            nc.scalar.dma_start(wk[:], wk_e[:])
            nc.scalar.dma_start(id128[:], id_e[:])
            nc.scalar.dma_start(madd[:], ma_e[:])
            nc.gpsimd.dma_start(wv[:], wv_e[:])
            nc.gpsimd.dma_start(bvb[:], bvb_e[:])
            nc.sync.dma_start(wo[:], wo_e[:])
            # ones columns of the V slots (col 64 of each 65-wide slot)
            v1r = v1.rearrange("p t (h c) -> p t h c", c=VS)
            for t in range(NT):
                nc.any.memset(v1r[:, t, :, 64:65], 1.0)

            with tc.tile_pool(name="pp", bufs=3, space="PSUM") as pp, \
                 tc.tile_pool(name="sc", bufs=4) as sc, \
                 tc.tile_pool(name="scp", bufs=3, space="PSUM") as scp, \
                 tc.tile_pool(name="cxp", bufs=1, space="PSUM") as cxp, \
                 tc.tile_pool(name="ep", bufs=5) as ep, \
                 tc.tile_pool(name="npl", bufs=2) as npl, \
                 tc.tile_pool(name="ysp", bufs=2) as ysp:
                pending = []

                def flush_one():
                    dst, raw, cos_ap, sin_ap = pending.pop(0)
                    pq = pp.tile([P, 512], f32, tag="ps", name="pq")
                    nc.tensor.matmul(pq[:], p128[:], raw[:],
                                     start=True, stop=True)
                    t1 = sc.tile([P, 512], f16, tag="t1", name="t1")
                    nc.vector.tensor_mul(t1[:], raw[:], cos_ap)
                    t2 = sc.tile([P, 512], f16, tag="t2", name="t2")
                    nc.vector.tensor_mul(t2[:], pq[:], sin_ap)
                    nc.vector.tensor_add(dst, t1[:], t2[:])

                def rope_chain(dst, w_sb, bias_col, t, csl):
                    ps = pp.tile([P, 512], f32, tag="ps", name="ps")
                    for k in range(NT):
                        nc.tensor.matmul(ps[:], w_sb[:, k, P * t:P * (t + 1)],
                                         xT[:, k, csl], start=(k == 0),
                                         stop=(k == NT - 1))
                    # psum->sbuf f16 with fused per-partition bias (ScalarE,
                    # which is otherwise idle during the projection phase)
                    raw = sc.tile([P, 512], f16, tag="raw", name="raw")
                    nc.scalar.activation(raw[:], ps[:], AF.Identity,
                                         bias=bias_col)
                    pending.append((dst, raw, cos[:, csl], sin[:, csl]))
                    if len(pending) > 1:
                        flush_one()

                def emit_q(t):
                    for n in range(2):
                        csl = slice(512 * n, 512 * (n + 1))
                        rope_chain(qt[:, t, csl], wq, bqt[:, t:t + 1], t, csl)

                def emit_k(t):
                    for n in range(2):
                        csl = slice(512 * n, 512 * (n + 1))
                        rope_chain(kt[:, t, csl], wk, bkt[:, t:t + 1], t, csl)

                def emit_v(t):
                    # V s-tile t: natural [s, dout_half] into 65-wide slots
                    ssl = slice(P * t, P * (t + 1))
                    vp = pp.tile([P, 512], f32, tag="ps", name="vp")
                    for k in range(NT):
                        nc.tensor.matmul(vp[:], xT[:, k, ssl], wv[:, k, :],
                                         start=(k == 0), stop=(k == NT - 1))
                    nc.vector.tensor_add(
                        v1r[:, t, :, 0:64],
                        vp.rearrange("p (h c) -> p h c", c=64),
                        bvb.rearrange("p (h c) -> p h c", c=64))

                # ---- attention: one flat (h, j) pipeline ----
                es = {}
                cxs = {}

                def filler():
                    # tiny independent matmul: keeps the PE active (HAM at
                    # K=8) while ctx waits on ScalarE exps
                    fp = pp.tile([P, 512], f32, tag="ps", name="fil")
                    nc.tensor.matmul(fp[:], warm[:, 0:P], warm[:],
                                     start=True, stop=True)

                def emit_scores(h, j):
                    # scores s[kv, q] for kv-block j; the additive -240
                    # triangle matmul masks the diagonal block before exp
                    th, hp = h // 2, h % 2
                    rsl = slice(64 * hp, 64 * hp + 64)
                    e = ep.tile([P, S], f16, tag="e", name=f"e{h}_{j}")
                    ksl = slice(P * j, P * (j + 1))
                    if j < 4:
                        wa = 512 - P * j
                        sA = scp.tile([P, 512], f32, tag="s",
                                      name=f"sA{h}_{j}")
                        nc.tensor.matmul(sA[:, 0:wa], kt[rsl, th, ksl],
                                         qt[rsl, th, P * j:512],
                                         start=True, stop=False,
                                         skip_group_check=True)
                        nc.tensor.matmul(sA[:, 0:P], id128[:], madd[:],
                                         start=False, stop=True,
                                         skip_group_check=True)
                        nc.scalar.activation(e[:, 0:wa], sA[:, 0:wa],
                                             AF.Exp, scale=0.125)
                        sB = scp.tile([P, 512], f32, tag="s",
                                      name=f"sB{h}_{j}")
                        nc.tensor.matmul(sB[:], kt[rsl, th, ksl],
                                         qt[rsl, th, 512:1024],
                                         start=True, stop=True,
                                         skip_group_check=True)
                        nc.scalar.activation(e[:, wa:wa + 512], sB[:],
                                             AF.Exp, scale=0.125)
                    else:
                        N = S - P * j
                        sA = scp.tile([P, 512], f32, tag="s",
                                      name=f"sA{h}_{j}")
                        nc.tensor.matmul(sA[:, 0:N], kt[rsl, th, ksl],
                                         qt[rsl, th, P * j:1024],
                                         start=True, stop=False,
                                         skip_group_check=True)
                        nc.tensor.matmul(sA[:, 0:P], id128[:], madd[:],
                                         start=False, stop=True,
                                         skip_group_check=True)
                        nc.scalar.activation(e[:, 0:N], sA[:, 0:N],
                                             AF.Exp, scale=0.125)
                    es[(h, j)] = e

                def emit_ctx(h, j):
                    e = es.pop((h, j))
                    if j == 0:
                        cxs[h] = (
                            cxp.tile([VS, 512], f32, tag="cxL",
                                     name=f"cxL{h}"),
                            cxp.tile([VS, 512], f32, tag="cxR",
                                     name=f"cxR{h}"),
                        )
                    cxL, cxR = cxs[h]
                    slot = v1[:, j, VS * h:VS * h + VS]
                    if j < 4:
                        wa = 512 - P * j
                        nc.tensor.matmul(cxL[:, P * j:512], slot,
                                         e[:, 0:wa], start=(j == 0),
                                         stop=(j == 3))
                        nc.tensor.matmul(cxR[:], slot, e[:, wa:wa + 512],
                                         start=(j == 0), stop=(j == 7))
                    else:
                        N = S - P * j
                        nc.tensor.matmul(cxR[:, P * j - 512:512], slot,
                                         e[:, 0:N], start=False,
                                         stop=(j == 7))

                def emit_norm(h):
                    # copy cx psum -> sbuf first: frees the psum banks for
                    # the next head after ~1us instead of after the whole
                    # normalize chain
                    th, hp = h // 2, h % 2
                    rsl = slice(64 * hp, 64 * hp + 64)
                    cxL, cxR = cxs.pop(h)
                    sxL = npl.tile([VS, 512], f32, tag="sxL", name="sxL")
                    sxR = npl.tile([VS, 512], f32, tag="sxR", name="sxR")
                    nc.vector.tensor_copy(sxL[:], cxL[:])
                    nc.vector.tensor_copy(sxR[:], cxR[:])
                    rrL = npl.tile([1, 512], f32, tag="rrL", name="rrL")
                    rrR = npl.tile([1, 512], f32, tag="rrR", name="rrR")
                    nc.vector.reciprocal_approx_fast(rrL[:], sxL[64:65, :])
                    nc.vector.reciprocal_approx_fast(rrR[:], sxR[64:65, :])
                    rbL = npl.tile([64, 512], f32, tag="rbL", name="rbL")
                    rbR = npl.tile([64, 512], f32, tag="rbR", name="rbR")
                    nc.gpsimd.partition_broadcast(rbL[:], rrL[:], channels=64)
                    nc.gpsimd.partition_broadcast(rbR[:], rrR[:], channels=64)
                    nc.gpsimd.tensor_mul(cn[rsl, th, 0:512], sxL[0:64, :],
                                         rbL[:])
                    nc.gpsimd.tensor_mul(cn[rsl, th, 512:1024], sxR[0:64, :],
                                         rbR[:])

                # Dummy matmuls at the head of the PE queue: keep the PE
                # array busy while the first DMAs land so the HAM clock
                # gate opens before the real chains start.
                warm = sc.tile([P, 512], f16, tag="warm", name="warm")
                nc.vector.memset(warm[:], 0.0)
                for i in range(28):
                    wp = pp.tile([P, 512], f32, tag="ps", name="wp")
                    nc.tensor.matmul(wp[:], warm[:, 0:P], warm[:],
                                     start=True, stop=True)

                # Projections: Q first (xT + wq + consts land first), then
                # V (wv), then kt tiles 0/1; kt tiles 2/3 are injected into
                # the flat attention pipeline below.
                for t in range(QT_T):
                    emit_q(t)
                for t in range(NT):
                    emit_v(t)
                emit_k(0)
                emit_k(1)

                # Flat attention pipeline: ctx lags scores by LAG steps so
                # the next head's (independent) score matmuls fill the PE
                # queue while this head's ctx waits on its exps.
                LAG = 4
                steps = [(h, j) for h in range(HG) for j in range(NT)]
                for pos in range(len(steps) + LAG):
                    if pos == 8:
                        emit_k(2)
                    elif pos == 16:
                        emit_k(3)
                        while pending:
                            flush_one()
                    if pos < len(steps):
                        emit_scores(*steps[pos])
                    if pos >= LAG:
                        h, j = steps[pos - LAG]
                        if pos >= 20:
                            filler()
                        emit_ctx(h, j)
                        if j == NT - 1:
                            emit_norm(h)

                # ---- partial out-projection ----
                for i in range(NT):
                    ys = ysp.tile([P, S], f16, tag="ys", name=f"ys{i}")
                    for n in range(2):
                        yp = pp.tile([P, 512], f32, tag="ps",
                                     name=f"yp{i}_{n}")
                        csl = slice(512 * n, 512 * (n + 1))
                        for t in range(QT_T):
                            nc.tensor.matmul(yp[:],
                                             cn[:, t, P * i:P * (i + 1)],
                                             wo[:, t, csl],
                                             start=(t == 0),
                                             stop=(t == QT_T - 1))
                        if n == 0:
                            nc.vector.tensor_copy(ys[:, csl], yp[:])
                        else:
                            nc.scalar.copy(ys[:, csl], yp[:])
                    q_eng = (nc.sync, nc.scalar, nc.gpsimd)[i % 3]
                    q_eng.dma_start(y_e[P * i:P * (i + 1), :], ys[:])

            if taps:
                for tn, tile_ap in (("qt", qt), ("kt", kt), ("v1", v1),
                                    ("cn", cn)):
                    nc.sync.dma_start(tap_ext[tn][:], tile_ap[:])

    nc.compile()
    return nc


def _host_tables():
    # RoPE tables, computed in float32 to match the reference's jnp path.
    pos = np.arange(S, dtype=np.float32)
    inv = np.exp(np.arange(0, Dh, 2, dtype=np.float32)
                 * np.float32(-np.log(10000.0) / Dh))          # [32]
    ang = pos[:, None] * inv[None, :]                          # [S, 32]
    sin = np.sin(ang).astype(np.float32)
    cos = np.cos(ang).astype(np.float32)
    # per-partition pattern for [2 heads x 64, s] transposed layout
    dd = np.arange(P) % Dh
    cosP = np.empty((P, S), np.float32)
    sinP = np.empty((P, S), np.float32)
    lo = dd < 32
    cosP[lo] = cos[:, dd[lo]].T
    sinP[lo] = -sin[:, dd[lo]].T
    cosP[~lo] = cos[:, dd[~lo] - 32].T
    sinP[~lo] = sin[:, dd[~lo] - 32].T
    return cosP.astype(np.float16), sinP.astype(np.float16)


def _perm128():
    p = np.zeros((P, P), np.float16)
    i = np.arange(P)
    p[i, i ^ 32] = np.float16(1.0)
    return p


def _tile_T(a):
    # [rows, D] -> [P, NT, rows]: partition-tiled transpose for SBUF layout
    rows = a.shape[0]
    return np.ascontiguousarray(a.T.reshape(NT, P, rows).transpose(1, 0, 2))


def _w_half(w, g):
    # Wx[:, 512g:512(g+1)] -> [P, NT, 512] in SBUF layout (contiguous)
    h = np.asarray(w, np.float16)[:, 512 * g:512 * (g + 1)]
    return np.ascontiguousarray(h.reshape(NT, P, 512).transpose(1, 0, 2))


def _wo_half(w, g):
    # Wo[512g:512(g+1), :] -> [P, QT_T, D] in SBUF layout (contiguous)
    h = np.asarray(w, np.float16)[512 * g:512 * (g + 1), :]
    return np.ascontiguousarray(h.reshape(QT_T, P, D).transpose(1, 0, 2))


def _b_half(b, g):
    h = np.asarray(b, np.float16).astype(np.float32)[512 * g:512 * (g + 1)]
    return np.ascontiguousarray(h.reshape(QT_T, P).T)


def make_in_maps(x, Wq, bq, Wk, bk, Wv, bv, Wo, bo):
    x = np.asarray(x, np.float16)
    cosP, sinP = _host_tables()
    r = np.arange(P)[:, None]
    c = np.arange(P)[None, :]
    madd = np.where(r > c, np.float16(-240.0), np.float16(0.0))
    id128 = np.eye(P, dtype=np.float16)
    p128 = _perm128()

    halves = []
    for g in range(2):
        halves.append({
            "wq": _w_half(Wq, g), "wk": _w_half(Wk, g),
            "wv": _w_half(Wv, g), "wo": _wo_half(Wo, g),
            "bqt": _b_half(bq, g), "bkt": _b_half(bk, g),
            "bvb": np.ascontiguousarray(np.broadcast_to(
                np.asarray(bv, np.float16)[512 * g:512 * (g + 1)]
                .reshape(1, 512), (P, 512))),
        })

    in_maps = []
    for core in range(N_CORES):
        b, g = core // 2, core % 2
        m = {
            "xt": _tile_T(x[b]),
            "cosk": cosP, "sink": sinP,
            "id128": id128, "madd": madd, "p128": p128,
        }
        m.update(halves[g])
        in_maps.append(m)
    return in_maps


def kernel(x, Wq, bq, Wk, bk, Wv, bv, Wo, bo):
    from concourse.bass_utils import run_bass_kernel_spmd

    with _lock:
        if "nc" not in _cache:
            _cache["nc"] = _build_program()
    nc = _cache["nc"]

    in_maps = make_in_maps(x, Wq, bq, Wk, bk, Wv, bv, Wo, bo)
    res = run_bass_kernel_spmd(nc, in_maps, list(range(N_CORES)))

    bo32 = np.asarray(bo, np.float16).astype(np.float32)
    out = np.empty((B, S, D), np.float16)
    for b in range(B):
        acc = res.results[2 * b]["y_sh"].astype(np.float32)
        acc += res.results[2 * b + 1]["y_sh"].astype(np.float32)
        out[b] = (acc + bo32).astype(np.float16)
    return out


# revision 12
# speedup vs baseline: 2.0263x; 2.0263x over previous
"""Trainium2 Bass kernel for CustomMultiHeadAttention (B=4, S=1024, D=1024, H=16, Dh=64).

Sharding: 8 cores = (batch b in 0..3) x (head-group g in 0..1).
Core (b, g) computes heads 8g..8g+7 of batch b over the FULL sequence:
Q/K/V projections use only the group's 512 columns of Wq/Wk/Wv, the
output projection contracts the group's 512 rows of Wo, producing a
partial [S, D] output; the host sums the two partials per batch (+bo).
Nothing is computed twice across cores, and per-core input DMA drops
to ~6.7 MB.

Pipeline notes:
 - QT/KT rope via permutation-matmul + DVE; psum evac with fused bias
   on ScalarE (idle during the projection phase).
 - The causal mask is an additive PE matmul: identity^T @ (-240
   triangle) accumulated into the diagonal 128-col block of each score
   chunk before the exp, so no vector/gpsimd engine ever touches the
   mask (engine-queue head-of-line blocking killed a previous variant).
 - Attention runs as ONE flat (head, kv-block) software pipeline with
   ctx lagging scores by 4 steps, so the next head's score matmuls fill
   the PE queue while ctx waits on ScalarE exps; filler matmuls keep
   the HAM clock gate at K=8 through the ScalarE-bound stretch.
 - normalize: cx psum is copied to SBUF immediately (frees the psum
   bank for the next head), reciprocals on DVE, partition-broadcasts
   and the cn multiplies on GpSimd.
"""

import threading

import numpy as np

B, S, D, H, Dh = 4, 1024, 1024, 16, 64
P = 128
N_CORES = 8
NT = D // P        # 8 tiles along the model dim
HG = 8             # heads per core
QT_T = 4           # qt/kt dout tiles per core (2 heads each)
VS = 65            # V slot width: [V(64) | ones(1)] per head

_cache = {}
_lock = threading.Lock()


def _build_program(taps=False):
    import concourse.bass as bass  # noqa: F401
    import concourse.mybir as mybir
    import concourse.tile as tile
    from concourse import bacc

    dt = mybir.dt
    f16, f32 = dt.float16, dt.float32
    AF = mybir.ActivationFunctionType

    nc = bacc.Bacc("TRN2", target_bir_lowering=False, debug=False,
                   num_devices=N_CORES)

    def ein(name, shape):
        return nc.dram_tensor(name, shape, f16, kind="ExternalInput").ap()

    xt_e = ein("xt", [P, NT, S])          # x[b]^T, host-transposed
    wq_e = ein("wq", [P, NT, 512])        # Wq[:, half], host-tiled
    wk_e = ein("wk", [P, NT, 512])
    wv_e = ein("wv", [P, NT, 512])
    wo_e = ein("wo", [P, QT_T, D])        # Wo[half, :], host-tiled
    bqt_e = nc.dram_tensor("bqt", [P, QT_T], f32, kind="ExternalInput").ap()
    bkt_e = nc.dram_tensor("bkt", [P, QT_T], f32, kind="ExternalInput").ap()
    bvb_e = ein("bvb", [P, 512])          # bv[half] broadcast across parts
    cos_e = ein("cosk", [P, S])
    sin_e = ein("sink", [P, S])
    id_e = ein("id128", [P, P])           # identity (mask-add stationary)
    ma_e = ein("madd", [P, P])            # -240 strict-lower triangle
    p128_e = ein("p128", [P, P])          # rope xor-32 permutation
    y_e = nc.dram_tensor("y_sh", [S, D], f16, kind="ExternalOutput").ap()
    tap_ext = {}
    if taps:
        for tn, shape in (("qt", [P, QT_T, S]), ("kt", [P, QT_T, S]),
                          ("v1", [P, NT, HG * VS]), ("cn", [P, QT_T, S])):
            tap_ext[tn] = nc.dram_tensor("dbg_" + tn, shape, f16,
                                         kind="ExternalOutput").ap()

    with tile.TileContext(nc) as tc:
        from contextlib import ExitStack
        with ExitStack() as ctx:
            big = ctx.enter_context(tc.tile_pool(name="big", bufs=1))

            xT = big.tile([P, NT, S], f16, tag="xT")
            wq = big.tile([P, NT, 512], f16, tag="wq")
            wk = big.tile([P, NT, 512], f16, tag="wk")
            wv = big.tile([P, NT, 512], f16, tag="wv")
            wo = big.tile([P, QT_T, D], f16, tag="wo")
            bqt = big.tile([P, QT_T], f32, tag="bqt")
            bkt = big.tile([P, QT_T], f32, tag="bkt")
            bvb = big.tile([P, 512], f16, tag="bvb")
            qt = big.tile([P, QT_T, S], f16, tag="qt")    # rope'd Q^T
            kt = big.tile([P, QT_T, S], f16, tag="kt")    # rope'd K^T
            v1 = big.tile([P, NT, HG * VS], f16, tag="v1")
            cn = big.tile([P, QT_T, S], f16, tag="cn")    # normalized ctx^T
            cos = big.tile([P, S], f16, tag="cos")
            sin = big.tile([P, S], f16, tag="sin")
            id128 = big.tile([P, P], f16, tag="id128")
            madd = big.tile([P, P], f16, tag="madd")
            p128 = big.tile([P, P], f16, tag="p128")

            # ---- input DMAs ----
            # Three queues pull in parallel; per-queue order matches first
            # use. Every tensor is host-packed to >=2KB contiguous lines.
            for t, e in ((cos, cos_e), (sin, sin_e), (bqt, bqt_e),
                         (bkt, bkt_e), (p128, p128_e)):
                nc.scalar.dma_start(t[:], e[:])
            nc.gpsimd.dma_start(wq[:, 0:4, :], wq_e[:, 0:4, :])
            nc.sync.dma_start(xT[:, 0:4, :], xt_e[:, 0:4, :])
            nc.gpsimd.dma_start(wq[:, 4:8, :], wq_e[:, 4:8, :])
            nc.sync.dma_start(xT[:, 4:8, :], xt_e[:, 4:8, :])
            nc.scalar.dma_start(wk[:], wk_e[:])
            nc.scalar.dma_start(id128[:], id_e[:])
            nc.scalar.dma_start(madd[:], ma_e[:])
            nc.gpsimd.dma_start(wv[:], wv_e[:])
            nc.gpsimd.dma_start(bvb[:], bvb_e[:])
            nc.sync.dma_start(wo[:], wo_e[:])
            # ones columns of the V slots (col 64 of each 65-wide slot)
            v1r = v1.rearrange("p t (h c) -> p t h c", c=VS)
            for t in range(NT):
                nc.any.memset(v1r[:, t, :, 64:65], 1.0)

            with tc.tile_pool(name="pp", bufs=3, space="PSUM") as pp, \
                 tc.tile_pool(name="sc", bufs=4) as sc, \
                 tc.tile_pool(name="scp", bufs=3, space="PSUM") as scp, \
                 tc.tile_pool(name="cxp", bufs=1, space="PSUM") as cxp, \
                 tc.tile_pool(name="ep", bufs=5) as ep, \
                 tc.tile_pool(name="npl", bufs=3) as npl, \
                 tc.tile_pool(name="ysp", bufs=2) as ysp:
                pending = []

                def flush_one():
                    dst, raw, cos_ap, sin_ap = pending.pop(0)
                    pq = pp.tile([P, 512], f32, tag="ps", name="pq")
                    nc.tensor.matmul(pq[:], p128[:], raw[:],
                                     start=True, stop=True)
                    t1 = sc.tile([P, 512], f16, tag="t1", name="t1")
                    nc.vector.tensor_mul(t1[:], raw[:], cos_ap)
                    t2 = sc.tile([P, 512], f16, tag="t2", name="t2")
                    nc.vector.tensor_mul(t2[:], pq[:], sin_ap)
                    nc.vector.tensor_add(dst, t1[:], t2[:])

                def rope_chain(dst, w_sb, bias_col, t, csl):
                    ps = pp.tile([P, 512], f32, tag="ps", name="ps")
                    for k in range(NT):
                        nc.tensor.matmul(ps[:], w_sb[:, k, P * t:P * (t + 1)],
                                         xT[:, k, csl], start=(k == 0),
                                         stop=(k == NT - 1))
                    # psum->sbuf f16 with fused per-partition bias (ScalarE,
                    # which is otherwise idle during the projection phase)
                    raw = sc.tile([P, 512], f16, tag="raw", name="raw")
                    nc.scalar.activation(raw[:], ps[:], AF.Identity,
                                         bias=bias_col)
                    pending.append((dst, raw, cos[:, csl], sin[:, csl]))
                    if len(pending) > 1:
                        flush_one()

                def emit_q(t):
                    for n in range(2):
                        csl = slice(512 * n, 512 * (n + 1))
                        rope_chain(qt[:, t, csl], wq, bqt[:, t:t + 1], t, csl)

                def emit_k(t):
                    for n in range(2):
                        csl = slice(512 * n, 512 * (n + 1))
                        rope_chain(kt[:, t, csl], wk, bkt[:, t:t + 1], t, csl)

                def emit_v(t):
                    # V s-tile t: natural [s, dout_half] into 65-wide slots
                    ssl = slice(P * t, P * (t + 1))
                    vp = pp.tile([P, 512], f32, tag="ps", name="vp")
                    for k in range(NT):
                        nc.tensor.matmul(vp[:], xT[:, k, ssl], wv[:, k, :],
                                         start=(k == 0), stop=(k == NT - 1))
                    nc.vector.tensor_add(
                        v1r[:, t, :, 0:64],
                        vp.rearrange("p (h c) -> p h c", c=64),
                        bvb.rearrange("p (h c) -> p h c", c=64))

                # ---- attention: one flat (h, j) pipeline ----
                es = {}
                cxs = {}

                def filler():
                    # tiny independent matmul: keeps the PE active (HAM at
                    # K=8) while ctx waits on ScalarE exps
                    fp = pp.tile([P, 512], f32, tag="ps", name="fil")
                    nc.tensor.matmul(fp[:], warm[:, 0:P], warm[:],
                                     start=True, stop=True)

                def emit_scores(h, j):
                    # scores s[kv, q] for kv-block j; the additive -240
                    # triangle matmul masks the diagonal block before exp
                    th, hp = h // 2, h % 2
                    rsl = slice(64 * hp, 64 * hp + 64)
                    e = ep.tile([P, S], f16, tag="e", name=f"e{h}_{j}")
                    ksl = slice(P * j, P * (j + 1))
                    if j < 4:
                        wa = 512 - P * j
                        sA = scp.tile([P, 512], f32, tag="s",
                                      name=f"sA{h}_{j}")
                        nc.tensor.matmul(sA[:, 0:wa], kt[rsl, th, ksl],
                                         qt[rsl, th, P * j:512],
                                         start=True, stop=False,
                                         skip_group_check=True)
                        nc.tensor.matmul(sA[:, 0:P], id128[:], madd[:],
                                         start=False, stop=True,
                                         skip_group_check=True)
                        nc.scalar.activation(e[:, 0:wa], sA[:, 0:wa],
                                             AF.Exp, scale=0.125)
                        sB = scp.tile([P, 512], f32, tag="s",
                                      name=f"sB{h}_{j}")
                        nc.tensor.matmul(sB[:], kt[rsl, th, ksl],
                                         qt[rsl, th, 512:1024],
                                         start=True, stop=True,
                                         skip_group_check=True)
                        nc.scalar.activation(e[:, wa:wa + 512], sB[:],
                                             AF.Exp, scale=0.125)
                    else:
                        N = S - P * j
                        sA = scp.tile([P, 512], f32, tag="s",
                                      name=f"sA{h}_{j}")
                        nc.tensor.matmul(sA[:, 0:N], kt[rsl, th, ksl],
                                         qt[rsl, th, P * j:1024],
                                         start=True, stop=False,
                                         skip_group_check=True)
                        nc.tensor.matmul(sA[:, 0:P], id128[:], madd[:],
                                         start=False, stop=True,
                                         skip_group_check=True)
                        nc.scalar.activation(e[:, 0:N], sA[:, 0:N],
                                             AF.Exp, scale=0.125)
                    es[(h, j)] = e

                def emit_ctx(h, j):
                    e = es.pop((h, j))
                    if j == 0:
                        cxs[h] = (
                            cxp.tile([VS, 512], f32, tag="cxL",
                                     name=f"cxL{h}"),
                            cxp.tile([VS, 512], f32, tag="cxR",
                                     name=f"cxR{h}"),
                        )
                    cxL, cxR = cxs[h]
                    slot = v1[:, j, VS * h:VS * h + VS]
                    if j < 4:
                        wa = 512 - P * j
                        nc.tensor.matmul(cxL[:, P * j:512], slot,
                                         e[:, 0:wa], start=(j == 0),
                                         stop=(j == 3))
                        nc.tensor.matmul(cxR[:], slot, e[:, wa:wa + 512],
                                         start=(j == 0), stop=(j == 7))
                    else:
                        N = S - P * j
                        nc.tensor.matmul(cxR[:, P * j - 512:512], slot,
                                         e[:, 0:N], start=False,
                                         stop=(j == 7))

                def emit_norm(h):
                    # copy cx psum -> sbuf first: frees the psum banks for
                    # the next head after ~1us instead of after the whole
                    # normalize chain
                    th, hp = h // 2, h % 2
                    rsl = slice(64 * hp, 64 * hp + 64)
                    cxL, cxR = cxs.pop(h)
                    sxL = npl.tile([VS, 512], f32, tag="sxL", name="sxL")
                    sxR = npl.tile([VS, 512], f32, tag="sxR", name="sxR")
                    nc.vector.tensor_copy(sxL[:], cxL[:])
                    nc.vector.tensor_copy(sxR[:], cxR[:])
                    rrL = npl.tile([1, 512], f32, tag="rrL", name="rrL")
                    rrR = npl.tile([1, 512], f32, tag="rrR", name="rrR")
                    nc.vector.reciprocal_approx_fast(rrL[:], sxL[64:65, :])
                    nc.vector.reciprocal_approx_fast(rrR[:], sxR[64:65, :])
                    rbL = npl.tile([64, 512], f32, tag="rbL", name="rbL")
                    rbR = npl.tile([64, 512], f32, tag="rbR", name="rbR")
                    nc.gpsimd.partition_broadcast(rbL[:], rrL[:], channels=64)
                    nc.gpsimd.partition_broadcast(rbR[:], rrR[:], channels=64)
                    nc.vector.tensor_mul(cn[rsl, th, 0:512], sxL[0:64, :],
                                         rbL[:])
                    nc.vector.tensor_mul(cn[rsl, th, 512:1024], sxR[0:64, :],
                                         rbR[:])

                # Dummy matmuls at the head of the PE queue: keep the PE
                # array busy while the first DMAs land so the HAM clock
                # gate opens before the real chains start.
                warm = sc.tile([P, 512], f16, tag="warm", name="warm")
                nc.vector.memset(warm[:], 0.0)
                for i in range(28):
                    wp = pp.tile([P, 512], f32, tag="ps", name="wp")
                    nc.tensor.matmul(wp[:], warm[:, 0:P], warm[:],
                                     start=True, stop=True)

                # Projections: Q first (xT + wq + consts land first), then
                # V (wv), then kt tiles 0/1; kt tiles 2/3 are injected into
                # the flat attention pipeline below.
                for t in range(QT_T):
                    emit_q(t)
                for t in range(NT):
                    emit_v(t)
                emit_k(0)
                emit_k(1)

                # Flat attention pipeline: ctx lags scores by LAG steps so
                # the next head's (independent) score matmuls fill the PE
                # queue while this head's ctx waits on its exps.
                LAG = 4
                steps = [(h, j) for h in range(HG) for j in range(NT)]
                for pos in range(len(steps) + LAG):
                    if pos == 8:
                        emit_k(2)
                    elif pos == 16:
                        emit_k(3)
                        while pending:
                            flush_one()
                    if pos < len(steps):
                        emit_scores(*steps[pos])
                    if pos >= LAG:
                        h, j = steps[pos - LAG]
                        if pos >= 20:
                            filler()
                        emit_ctx(h, j)
                        if j == NT - 1:
                            emit_norm(h)

                # ---- partial out-projection ----
                for i in range(NT):
                    ys = ysp.tile([P, S], f16, tag="ys", name=f"ys{i}")
                    for n in range(2):
                        yp = pp.tile([P, 512], f32, tag="ps",
                                     name=f"yp{i}_{n}")
                        csl = slice(512 * n, 512 * (n + 1))
                        for t in range(QT_T):
                            nc.tensor.matmul(yp[:],
                                             cn[:, t, P * i:P * (i + 1)],
                                             wo[:, t, csl],
                                             start=(t == 0),
                                             stop=(t == QT_T - 1))
                        if n == 0:
                            nc.vector.tensor_copy(ys[:, csl], yp[:])
                        else:
                            nc.scalar.copy(ys[:, csl], yp[:])
                    q_eng = (nc.sync, nc.scalar, nc.gpsimd)[i % 3]
                    q_eng.dma_start(y_e[P * i:P * (i + 1), :], ys[:])

            if taps:
                for tn, tile_ap in (("qt", qt), ("kt", kt), ("v1", v1),
                                    ("cn", cn)):
                    nc.sync.dma_start(tap_ext[tn][:], tile_ap[:])

    nc.compile()
    return nc


def _host_tables():
    # RoPE tables, computed in float32 to match the reference's jnp path.
    pos = np.arange(S, dtype=np.float32)
    inv = np.exp(np.arange(0, Dh, 2, dtype=np.float32)
                 * np.float32(-np.log(10000.0) / Dh))          # [32]
    ang = pos[:, None] * inv[None, :]                          # [S, 32]
    sin = np.sin(ang).astype(np.float32)
    cos = np.cos(ang).astype(np.float32)
    # per-partition pattern for [2 heads x 64, s] transposed layout
    dd = np.arange(P) % Dh
    cosP = np.empty((P, S), np.float32)
    sinP = np.empty((P, S), np.float32)
    lo = dd < 32
    cosP[lo] = cos[:, dd[lo]].T
    sinP[lo] = -sin[:, dd[lo]].T
    cosP[~lo] = cos[:, dd[~lo] - 32].T
    sinP[~lo] = sin[:, dd[~lo] - 32].T
    return cosP.astype(np.float16), sinP.astype(np.float16)


def _perm128():
    p = np.zeros((P, P), np.float16)
    i = np.arange(P)
    p[i, i ^ 32] = np.float16(1.0)
    return p


def _tile_T(a):
    # [rows, D] -> [P, NT, rows]: partition-tiled transpose for SBUF layout
    rows = a.shape[0]
    return np.ascontiguousarray(a.T.reshape(NT, P, rows).transpose(1, 0, 2))


def _w_half(w, g):
    # Wx[:, 512g:512(g+1)] -> [P, NT, 512] in SBUF layout (contiguous)
    h = np.asarray(w, np.float16)[:, 512 * g:512 * (g + 1)]
    return np.ascontiguousarray(h.reshape(NT, P, 512).transpose(1, 0, 2))


def _wo_half(w, g):
    # Wo[512g:512(g+1), :] -> [P, QT_T, D] in SBUF layout (contiguous)
    h = np.asarray(w, np.float16)[512 * g:512 * (g + 1), :]
    return np.ascontiguousarray(h.reshape(QT_T, P, D).transpose(1, 0, 2))


def _b_half(b, g):
    h = np.asarray(b, np.float16).astype(np.float32)[512 * g:512 * (g + 1)]
    return np.ascontiguousarray(h.reshape(QT_T, P).T)


def make_in_maps(x, Wq, bq, Wk, bk, Wv, bv, Wo, bo):
    x = np.asarray(x, np.float16)
    cosP, sinP = _host_tables()
    r = np.arange(P)[:, None]
    c = np.arange(P)[None, :]
    # -120/8 = -15 per masked logit: e^-15 * 64 masked terms is ~2e-5 of
    # the softmax denominator (exact-enough mask); stays well inside the
    # ScalarE exp spline's input range, unlike larger constants.
    madd = np.where(r > c, np.float16(-120.0), np.float16(0.0))
    id128 = np.eye(P, dtype=np.float16)
    p128 = _perm128()

    halves = []
    for g in range(2):
        halves.append({
            "wq": _w_half(Wq, g), "wk": _w_half(Wk, g),
            "wv": _w_half(Wv, g), "wo": _wo_half(Wo, g),
            "bqt": _b_half(bq, g), "bkt": _b_half(bk, g),
            "bvb": np.ascontiguousarray(np.broadcast_to(
                np.asarray(bv, np.float16)[512 * g:512 * (g + 1)]
                .reshape(1, 512), (P, 512))),
        })

    in_maps = []
    for core in range(N_CORES):
        b, g = core // 2, core % 2
        m = {
            "xt": _tile_T(x[b]),
            "cosk": cosP, "sink": sinP,
            "id128": id128, "madd": madd, "p128": p128,
        }
        m.update(halves[g])
        in_maps.append(m)
    return in_maps


def kernel(x, Wq, bq, Wk, bk, Wv, bv, Wo, bo):
    from concourse.bass_utils import run_bass_kernel_spmd

    with _lock:
        if "nc" not in _cache:
            _cache["nc"] = _build_program()
    nc = _cache["nc"]

    in_maps = make_in_maps(x, Wq, bq, Wk, bk, Wv, bv, Wo, bo)
    res = run_bass_kernel_spmd(nc, in_maps, list(range(N_CORES)))

    bo32 = np.asarray(bo, np.float16).astype(np.float32)
    out = np.empty((B, S, D), np.float16)
    for b in range(B):
        acc = res.results[2 * b]["y_sh"].astype(np.float32)
        acc += res.results[2 * b + 1]["y_sh"].astype(np.float32)
        out[b] = (acc + bo32).astype(np.float16)
    return out
